# revision 42
# baseline (speedup 1.0000x reference)
"""MoE layer (E=8 experts, top-2 routing) on 8 Trainium2 NeuronCores.

Strategy (expert-parallel, per the sharding hint):
  - The gate (T x D @ D x E, softmax, top-2, renorm) is computed on the host
    in fp32; it is ~0.01% of the FLOPs.
  - Tokens are dispatched by expert id ("all-to-all" done host-side): core e
    receives the tokens routed to expert e (padded to a common capacity C),
    together with expert e's weights in bf16.
  - Each core runs a Bass/Tile kernel computing
        y = combine_weight * (gelu(x @ w1 + b1) @ w2 + b2)
    with bf16 matmuls (fp32 PSUM accumulation) on the PE array:
      * phase H: H^T tiles (feature-major) = w1-chunk^T.T @ x^T-chunk,
        so no on-device transposes are needed (w1 natural layout is lhsT).
      * phase Y: token-major Y = H^T-chunk.T @ w2-chunk, which makes the
        per-token combine weight a per-partition scalar.
  - Host "unshard" is two gathers + an add (each token has exactly 2 slots).
"""

import sys
import types

import numpy as np
import ml_dtypes

import concourse.bass as bass
import concourse.mybir as mybir
from concourse import bacc
from concourse.tile import TileContext
from concourse.bass_utils import run_bass_kernel_spmd


def _ensure_antenv_hooks():
    """bass_utils imports antenv.axon_hooks when BASS_TRACE is set; this image
    may lack it. Provide the registry (with the real ctypes NTFF hook when
    available) so tracing works instead of crashing."""
    try:
        import antenv.axon_hooks  # noqa: F401
        return
    except ImportError:
        pass
    if "antenv" not in sys.modules:
        try:
            import antenv  # noqa: F401
        except ImportError:
            sys.modules["antenv"] = types.ModuleType("antenv")
    hooks = types.ModuleType("antenv.axon_hooks")
    state = {"hook": None}
    hooks.set_axon_ntff_profile_hook = lambda h: state.__setitem__("hook", h)
    hooks.get_axon_ntff_profile_hook = lambda: state["hook"]
    sys.modules["antenv"].axon_hooks = hooks
    sys.modules["antenv.axon_hooks"] = hooks
    try:
        from trn_agent_boot.trn_boot import _ntff_profile_via_ctypes
        hook = _ntff_profile_via_ctypes("/opt/axon/libaxon_pjrt.so")
        if hook is not None:
            hooks.set_axon_ntff_profile_hook(hook)
    except Exception:
        pass


_ensure_antenv_hooks()

P = 128
D = 1024
F = 4096
E = 8
TOPK = 2
NBLK = 512

_BF16 = ml_dtypes.bfloat16

_nc_cache: dict = {}
LAST = None  # BassKernelResults of the most recent run (for test harness)


def _build_moe_core(C: int) -> bass.Bass:
    """One-core SPMD program: FFN for C tokens with resident bf16 weights."""
    dt = mybir.dt
    nc = bacc.Bacc("TRN2", target_bir_lowering=False, debug=False)
    KO = D // P    # 8 contraction chunks for x @ w1
    FO = F // P    # 32 contraction chunks for h @ w2
    DN = D // NBLK  # 2 output-column blocks of w2
    GELU = mybir.ActivationFunctionType.Gelu

    xt = nc.dram_tensor("xt", [D, C], dt.bfloat16, kind="ExternalInput")
    # w1 host-pretiled per-fo: w1t[fo, p, ko, j] = w1[ko*P+p, fo*P+j], so each
    # 256KB fo-tile is one contiguous-per-partition DMA and the PE can start
    # after the first tile instead of the full 8MB.
    w1t = nc.dram_tensor("w1t", [FO, P, KO, P], dt.bfloat16,
                         kind="ExternalInput")
    w2 = nc.dram_tensor("w2", [F, D], dt.bfloat16, kind="ExternalInput")
    # b1/sc pre-packed partition-major on host so each DMA is one contiguous
    # descriptor per partition (the rearranged 1-D loads were 4B-strided).
    b1p = nc.dram_tensor("b1p", [P, FO], dt.float32, kind="ExternalInput")
    b2r = nc.dram_tensor("b2r", [P, D], dt.float32, kind="ExternalInput")
    scp = nc.dram_tensor("scp", [P, C // P], dt.float32, kind="ExternalInput")
    y = nc.dram_tensor("y", [C, D], dt.float32, kind="ExternalOutput")

    # Uniform 512-token blocks: smaller N makes the per-matmul LDWEIGHTS
    # (~97ns, FWL off in this toolchain) stop hiding inside the matmul
    # streaming window, measured +94ns/matmul at N=256.
    blocks = []
    off = 0
    while off < C:
        size = min(NBLK, C - off)
        blocks.append((off, size))
        off += size

    xt_r = xt.rearrange("(ko p) c -> p ko c", p=P)

    with TileContext(nc) as tc:
        with (
            tc.tile_pool(name="w", bufs=1) as wpool,
            tc.tile_pool(name="xin", bufs=2) as xpool,
            tc.tile_pool(name="h", bufs=1) as hpool,
            tc.tile_pool(name="yout", bufs=2) as ypool,
            tc.tile_pool(name="ph", bufs=3, space="PSUM") as phpool,
            tc.tile_pool(name="py", bufs=4, space="PSUM") as pypool,
            tc.tile_pool(name="pw", bufs=1, space="PSUM") as pwpool,
        ):
            # DMA issue order is the startup critical path: x block 0 (one
            # fused 1MB transfer) and the first w1 fo-tile gate the first
            # matmul; b1 is needed by the first gelu shortly after; the rest
            # (remaining w1, b2, sc, w2) only gate later work.
            # x loads as per-ko chunk tiles: the opening H group's matmuls
            # then gate chunk-by-chunk, so real PE work starts as soon as
            # chunk 0 + the first w1 fo-tile land (~9us) and the HAM cold
            # period hides inside the DMA pacing stalls.
            def load_x_chunk(ko, n_off, n_size):
                xt_t = xpool.tile([P, NBLK], dt.bfloat16, tag=f"x_{ko}")
                nc.sync.dma_start(
                    xt_t[:, :n_size], xt_r[:, ko, n_off:n_off + n_size]
                )
                return xt_t

            def x_chunk(xts, ko):
                return xts[ko]

            # A short dummy burst primes the HAM busy-window while the first
            # x chunk is still in flight.
            warm = wpool.tile([P, NBLK], dt.bfloat16, tag="warm")
            nc.gpsimd.memset(warm[:], 0.0)
            pwarm = pwpool.tile([P, NBLK], dt.float32, tag="pw")
            NWARM = 6
            for i in range(NWARM):
                nc.tensor.matmul(
                    pwarm[:], warm[:, :P], warm[:],
                    start=(i == 0), stop=(i == NWARM - 1),
                )

            n_off0, n_size0 = blocks[0]
            xts0 = [load_x_chunk(0, n_off0, n_size0)]

            w1sb = []
            for fo in range(FO):
                t_ = wpool.tile([P, KO, P], dt.bfloat16, tag=f"w1_{fo}")
                nc.sync.dma_start(t_[:], w1t[fo])
                w1sb.append(t_)
                if fo == 0:
                    b1sb = wpool.tile([P, FO], dt.float32, tag="b1")
                    nc.sync.dma_start(b1sb[:], b1p[:])
                    for ko in range(1, KO):
                        xts0.append(load_x_chunk(ko, n_off0, n_size0))

            b2sb = wpool.tile([P, D], dt.float32, tag="b2")
            nc.sync.dma_start(b2sb[:], b2r[:])
            scsb = wpool.tile([P, C // P], dt.float32, tag="sc")
            nc.sync.dma_start(scsb[:], scp[:])

            # w2 is only needed once the first Y phase starts (~60us in), so a
            # single consolidated tile/DMA is fine and keeps the live
            # semaphore count (and the exit drain's split-wait storm) small.
            w2sb = wpool.tile([P, FO, D], dt.bfloat16, tag="w2")
            nc.sync.dma_start(w2sb[:], w2.rearrange("(fo p) d -> p fo d", p=P))

            for bi, (n_off, n_size) in enumerate(blocks):
                xts = xts0 if bi == 0 else [
                    load_x_chunk(ko, n_off, n_size) for ko in range(KO)
                ]

                # H^T[f, t] = sum_d w1[d, f] * x^T[d, t], then gelu(+b1).
                htile = hpool.tile([P, FO, NBLK], dt.bfloat16, tag="h")
                for fo in range(FO):
                    ph = phpool.tile([P, NBLK], dt.float32, tag="ph")
                    for ko in range(KO):
                        nc.tensor.matmul(
                            ph[:, :n_size],
                            w1sb[fo][:, ko, :],
                            x_chunk(xts, ko)[:, :n_size],
                            start=(ko == 0),
                            stop=(ko == KO - 1),
                        )
                    nc.scalar.activation(
                        htile[:, fo, :n_size], ph[:, :n_size], GELU,
                        bias=b1sb[:, fo:fo + 1], scale=1.0,
                    )

                # Y[t, d] = sum_f H[t, f] * w2[f, d]; scale per token.
                for tb in range(n_size // P):
                    tbg = (n_off + tb * P) // P
                    ytile = ypool.tile([P, D], dt.float32, tag="y")
                    # dn-outer: the d-half 0 epilogue (bias add, scale, store)
                    # overlaps the d-half 1 matmuls, so only ~1.7us of
                    # epilogue trails the very last matmul of the kernel.
                    for dn in range(DN):
                        py = pypool.tile([P, NBLK], dt.float32, tag="py")
                        for fo in range(FO):
                            nc.tensor.matmul(
                                py[:],
                                htile[:, fo, tb * P:(tb + 1) * P],
                                w2sb[:, fo, dn * NBLK:(dn + 1) * NBLK],
                                start=(fo == 0),
                                stop=(fo == FO - 1),
                            )
                        dsl = slice(dn * NBLK, (dn + 1) * NBLK)
                        nc.vector.tensor_add(
                            ytile[:, dsl], py[:], b2sb[:, dsl]
                        )
                        nc.vector.tensor_scalar_mul(
                            ytile[:, dsl], ytile[:, dsl], scsb[:, tbg:tbg + 1]
                        )
                        nc.sync.dma_start(
                            y[n_off + tb * P:n_off + (tb + 1) * P, dsl],
                            ytile[:, dsl],
                        )
    nc.compile()
    return nc


def _route(flat, gate_w, gate_b):
    """fp32 gate matching the reference: softmax, top-2, renormalize."""
    logits = flat @ gate_w + gate_b
    m = logits.max(axis=1, keepdims=True)
    p = np.exp(logits - m, dtype=np.float32)
    probs = p / p.sum(axis=1, keepdims=True)
    ti = np.argsort(-probs, axis=1, kind="stable")[:, :TOPK]
    tp = np.take_along_axis(probs, ti, axis=1)
    sw = tp / (tp.sum(axis=1, keepdims=True) + np.float32(1e-9))
    return ti.astype(np.int64), sw.astype(np.float32)


def _dispatch(ti):
    """Slot assignment: (token, k) pair -> (expert, position-in-expert)."""
    Tn = ti.shape[0]
    flat_e = ti.ravel()
    order = np.argsort(flat_e, kind="stable")
    cnt = np.bincount(flat_e, minlength=E)
    starts = np.concatenate([[0], np.cumsum(cnt)[:-1]])
    ranks = np.arange(Tn * TOPK) - starts[flat_e[order]]
    pos = np.empty(Tn * TOPK, np.int64)
    pos[order] = ranks
    return flat_e, pos, cnt, starts, order


def _gelu_exact(v):
    try:
        from scipy.special import erf
        return 0.5 * v * (1.0 + erf(v / np.sqrt(2.0)))
    except ImportError:  # tanh approximation fallback (overflow tokens only)
        return 0.5 * v * (1.0 + np.tanh(
            0.7978845608028654 * (v + 0.044715 * v ** 3)))


def kernel(**inputs) -> np.ndarray:
    global LAST
    x = np.asarray(inputs["x"], np.float32)
    gate_w = np.asarray(inputs["gate_w"], np.float32)
    gate_b = np.asarray(inputs["gate_b"], np.float32)
    w1 = np.asarray(inputs["w1"], np.float32)
    b1 = np.asarray(inputs["b1"], np.float32)
    w2 = np.asarray(inputs["w2"], np.float32)
    b2 = np.asarray(inputs["b2"], np.float32)

    B, S, D_ = x.shape
    flat = x.reshape(-1, D_)
    Tn = flat.shape[0]

    ti, sw = _route(flat, gate_w, gate_b)
    flat_e, pos, cnt, starts, order = _dispatch(ti)

    # Capacity factor 1.0: each core processes exactly T*K/E token slots (the
    # SPMD program is uniform, so every core pays the max expert's cost —
    # capping at the mean keeps the device critical path balanced). The few
    # overflow tokens of the hottest experts are combined on the host in fp32.
    cap = (Tn * TOPK // E + P - 1) // P * P
    C = ((int(cnt.max()) + P - 1) // P) * P
    C = max(min(C, cap), P)

    xT_bf = np.ascontiguousarray(flat.T).astype(_BF16)  # [D, T]
    sw_flat = sw.ravel()

    in_maps = []
    overflow = []
    for e in range(E):
        pairs_all = order[starts[e]:starts[e] + cnt[e]]
        pairs = pairs_all[:C]
        if cnt[e] > C:
            overflow.append((e, pairs_all[C:]))
        n_e = len(pairs)
        toks = pairs // TOPK
        xt_e = np.zeros((D, C), _BF16)
        xt_e[:, :n_e] = xT_bf[:, toks]
        sc_e = np.zeros((C,), np.float32)
        sc_e[:n_e] = sw_flat[pairs]
        KO, FO = D // P, F // P
        w1_tiled = np.ascontiguousarray(
            w1[e].astype(_BF16).reshape(KO, P, FO, P).transpose(2, 1, 0, 3)
        )
        in_maps.append({
            "xt": xt_e,
            "w1t": w1_tiled,
            "w2": w2[e].astype(_BF16),
            "b1p": np.ascontiguousarray(b1[e].reshape(F // P, P).T),
            "b2r": np.ascontiguousarray(
                np.broadcast_to(b2[e], (P, D))
            ).astype(np.float32),
            "scp": np.ascontiguousarray(sc_e.reshape(C // P, P).T),
        })

    nc = _nc_cache.get(C)
    if nc is None:
        nc = _build_moe_core(C)
        _nc_cache[C] = nc

    LAST = run_bass_kernel_spmd(nc, in_maps, core_ids=list(range(E)))
    Yall = np.stack([np.asarray(LAST.results[i]["y"]) for i in range(E)])

    # Combine: device slots via two gathers; host fp32 FFN for overflow.
    in_cap = pos < C
    contrib = np.zeros((Tn * TOPK, D_), np.float32)
    idx = np.nonzero(in_cap)[0]
    contrib[idx] = Yall[flat_e[idx], pos[idx]]
    out = contrib[0::TOPK] + contrib[1::TOPK]
    for e, over in overflow:
        toks = over // TOPK
        h = _gelu_exact(flat[toks] @ w1[e] + b1[e])
        y_e = h @ w2[e] + b2[e]
        out[toks] += sw_flat[over][:, None] * y_e
    return out.reshape(B, S, D_).astype(np.float32)


# revision 45
# speedup vs baseline: 1.0054x; 1.0054x over previous
"""MoE layer (E=8 experts, top-2 routing) on 8 Trainium2 NeuronCores.

Strategy (expert-parallel, per the sharding hint):
  - The gate (T x D @ D x E, softmax, top-2, renorm) is computed on the host
    in fp32; it is ~0.01% of the FLOPs.
  - Tokens are dispatched by expert id ("all-to-all" done host-side): core e
    receives the tokens routed to expert e (padded to a common capacity C),
    together with expert e's weights in bf16.
  - Each core runs a Bass/Tile kernel computing
        y = combine_weight * (gelu(x @ w1 + b1) @ w2 + b2)
    with bf16 matmuls (fp32 PSUM accumulation) on the PE array:
      * phase H: H^T tiles (feature-major) = w1-chunk^T.T @ x^T-chunk,
        so no on-device transposes are needed (w1 natural layout is lhsT).
      * phase Y: token-major Y = H^T-chunk.T @ w2-chunk, which makes the
        per-token combine weight a per-partition scalar.
  - Host "unshard" is two gathers + an add (each token has exactly 2 slots).
"""

import sys
import types

import numpy as np
import ml_dtypes

import concourse.bass as bass
import concourse.mybir as mybir
from concourse import bacc
from concourse.tile import TileContext
from concourse.bass_utils import run_bass_kernel_spmd


def _ensure_antenv_hooks():
    """bass_utils imports antenv.axon_hooks when BASS_TRACE is set; this image
    may lack it. Provide the registry (with the real ctypes NTFF hook when
    available) so tracing works instead of crashing."""
    try:
        import antenv.axon_hooks  # noqa: F401
        return
    except ImportError:
        pass
    if "antenv" not in sys.modules:
        try:
            import antenv  # noqa: F401
        except ImportError:
            sys.modules["antenv"] = types.ModuleType("antenv")
    hooks = types.ModuleType("antenv.axon_hooks")
    state = {"hook": None}
    hooks.set_axon_ntff_profile_hook = lambda h: state.__setitem__("hook", h)
    hooks.get_axon_ntff_profile_hook = lambda: state["hook"]
    sys.modules["antenv"].axon_hooks = hooks
    sys.modules["antenv.axon_hooks"] = hooks
    try:
        from trn_agent_boot.trn_boot import _ntff_profile_via_ctypes
        hook = _ntff_profile_via_ctypes("/opt/axon/libaxon_pjrt.so")
        if hook is not None:
            hooks.set_axon_ntff_profile_hook(hook)
    except Exception:
        pass


_ensure_antenv_hooks()

P = 128
D = 1024
F = 4096
E = 8
TOPK = 2
NBLK = 512

_BF16 = ml_dtypes.bfloat16

_nc_cache: dict = {}
LAST = None  # BassKernelResults of the most recent run (for test harness)


def _build_moe_core(C: int) -> bass.Bass:
    """One-core SPMD program: FFN for C tokens with resident bf16 weights."""
    dt = mybir.dt
    nc = bacc.Bacc("TRN2", target_bir_lowering=False, debug=False)
    KO = D // P    # 8 contraction chunks for x @ w1
    FO = F // P    # 32 contraction chunks for h @ w2
    DN = D // NBLK  # 2 output-column blocks of w2
    GELU = mybir.ActivationFunctionType.Gelu

    xt = nc.dram_tensor("xt", [D, C], dt.bfloat16, kind="ExternalInput")
    # w1 host-pretiled per-fo: w1t[fo, p, ko, j] = w1[ko*P+p, fo*P+j], so each
    # 256KB fo-tile is one contiguous-per-partition DMA and the PE can start
    # after the first tile instead of the full 8MB.
    w1t = nc.dram_tensor("w1t", [FO, P, KO, P], dt.bfloat16,
                         kind="ExternalInput")
    w2 = nc.dram_tensor("w2", [F, D], dt.bfloat16, kind="ExternalInput")
    # b1/sc pre-packed partition-major on host so each DMA is one contiguous
    # descriptor per partition (the rearranged 1-D loads were 4B-strided).
    b1p = nc.dram_tensor("b1p", [P, FO], dt.float32, kind="ExternalInput")
    b2r = nc.dram_tensor("b2r", [P, D], dt.float32, kind="ExternalInput")
    scp = nc.dram_tensor("scp", [P, C // P], dt.float32, kind="ExternalInput")
    y = nc.dram_tensor("y", [C, D], dt.float32, kind="ExternalOutput")

    # Uniform 512-token blocks: smaller N makes the per-matmul LDWEIGHTS
    # (~97ns, FWL off in this toolchain) stop hiding inside the matmul
    # streaming window, measured +94ns/matmul at N=256.
    blocks = []
    off = 0
    while off < C:
        size = min(NBLK, C - off)
        blocks.append((off, size))
        off += size

    xt_r = xt.rearrange("(ko p) c -> p ko c", p=P)

    with TileContext(nc) as tc:
        with (
            tc.tile_pool(name="w", bufs=1) as wpool,
            tc.tile_pool(name="xin", bufs=2) as xpool,
            tc.tile_pool(name="h", bufs=1) as hpool,
            tc.tile_pool(name="yout", bufs=2) as ypool,
            tc.tile_pool(name="ph", bufs=3, space="PSUM") as phpool,
            tc.tile_pool(name="py", bufs=4, space="PSUM") as pypool,
            tc.tile_pool(name="pw", bufs=1, space="PSUM") as pwpool,
        ):
            # DMA issue order is the startup critical path: x block 0 (one
            # fused 1MB transfer) and the first w1 fo-tile gate the first
            # matmul; b1 is needed by the first gelu shortly after; the rest
            # (remaining w1, b2, sc, w2) only gate later work.
            KH = KO // 2  # x blocks load as two half-tiles (finer DMA deps)

            def load_x_block(n_off, n_size):
                xa = xpool.tile([P, KH, NBLK], dt.bfloat16, tag="xa")
                nc.sync.dma_start(
                    xa[:, :, :n_size], xt_r[:, :KH, n_off:n_off + n_size]
                )
                xb = xpool.tile([P, KH, NBLK], dt.bfloat16, tag="xb")
                nc.sync.dma_start(
                    xb[:, :, :n_size], xt_r[:, KH:, n_off:n_off + n_size]
                )
                return xa, xb

            def x_chunk(xts, ko):
                return xts[0][:, ko, :] if ko < KH else xts[1][:, ko - KH, :]

            # Warm the PE's HAM clock gate during the startup DMA window
            # with dummy matmuls on zeroed SBUF. Sized to the (deterministic)
            # PE-start -> data-ready delta of ~8us: an idle gap > ~3.4us
            # before the real stream would re-throttle the clock and cost
            # ~3us of cold matmuls.
            warm = wpool.tile([P, NBLK], dt.bfloat16, tag="warm")
            nc.gpsimd.memset(warm[:], 0.0)
            pwarm = pwpool.tile([P, NBLK], dt.float32, tag="pw")
            NWARM = 24
            for i in range(NWARM):
                nc.tensor.matmul(
                    pwarm[:], warm[:, :P], warm[:],
                    start=(i == 0), stop=(i == NWARM - 1),
                )

            xts0 = load_x_block(*blocks[0])

            w1sb = []
            for fo in range(FO):
                t_ = wpool.tile([P, KO, P], dt.bfloat16, tag=f"w1_{fo}")
                nc.sync.dma_start(t_[:], w1t[fo])
                w1sb.append(t_)
                if fo == 0:
                    b1sb = wpool.tile([P, FO], dt.float32, tag="b1")
                    nc.sync.dma_start(b1sb[:], b1p[:])

            b2sb = wpool.tile([P, D], dt.float32, tag="b2")
            nc.sync.dma_start(b2sb[:], b2r[:])
            scsb = wpool.tile([P, C // P], dt.float32, tag="sc")
            nc.sync.dma_start(scsb[:], scp[:])

            # w2 is only needed once the first Y phase starts (~60us in), so a
            # single consolidated tile/DMA is fine and keeps the live
            # semaphore count (and the exit drain's split-wait storm) small.
            w2sb = wpool.tile([P, FO, D], dt.bfloat16, tag="w2")
            nc.sync.dma_start(w2sb[:], w2.rearrange("(fo p) d -> p fo d", p=P))

            for bi, (n_off, n_size) in enumerate(blocks):
                xts = xts0 if bi == 0 else load_x_block(n_off, n_size)

                # H^T[f, t] = sum_d w1[d, f] * x^T[d, t], then gelu(+b1).
                htile = hpool.tile([P, FO, NBLK], dt.bfloat16, tag="h")
                for fo in range(FO):
                    ph = phpool.tile([P, NBLK], dt.float32, tag="ph")
                    for ko in range(KO):
                        nc.tensor.matmul(
                            ph[:, :n_size],
                            w1sb[fo][:, ko, :],
                            x_chunk(xts, ko)[:, :n_size],
                            start=(ko == 0),
                            stop=(ko == KO - 1),
                        )
                    nc.scalar.activation(
                        htile[:, fo, :n_size], ph[:, :n_size], GELU,
                        bias=b1sb[:, fo:fo + 1], scale=1.0,
                    )

                # Y[t, d] = sum_f H[t, f] * w2[f, d]; scale per token.
                for tb in range(n_size // P):
                    tbg = (n_off + tb * P) // P
                    ytile = ypool.tile([P, D], dt.float32, tag="y")
                    # dn-outer: the d-half 0 epilogue (bias add, scale, store)
                    # overlaps the d-half 1 matmuls, so only ~1.7us of
                    # epilogue trails the very last matmul of the kernel.
                    for dn in range(DN):
                        py = pypool.tile([P, NBLK], dt.float32, tag="py")
                        for fo in range(FO):
                            nc.tensor.matmul(
                                py[:],
                                htile[:, fo, tb * P:(tb + 1) * P],
                                w2sb[:, fo, dn * NBLK:(dn + 1) * NBLK],
                                start=(fo == 0),
                                stop=(fo == FO - 1),
                            )
                        dsl = slice(dn * NBLK, (dn + 1) * NBLK)
                        nc.vector.tensor_add(
                            ytile[:, dsl], py[:], b2sb[:, dsl]
                        )
                        nc.vector.tensor_scalar_mul(
                            ytile[:, dsl], ytile[:, dsl], scsb[:, tbg:tbg + 1]
                        )
                        nc.sync.dma_start(
                            y[n_off + tb * P:n_off + (tb + 1) * P, dsl],
                            ytile[:, dsl],
                        )
    nc.compile()
    return nc


def _route(flat, gate_w, gate_b):
    """fp32 gate matching the reference: softmax, top-2, renormalize."""
    logits = flat @ gate_w + gate_b
    m = logits.max(axis=1, keepdims=True)
    p = np.exp(logits - m, dtype=np.float32)
    probs = p / p.sum(axis=1, keepdims=True)
    ti = np.argsort(-probs, axis=1, kind="stable")[:, :TOPK]
    tp = np.take_along_axis(probs, ti, axis=1)
    sw = tp / (tp.sum(axis=1, keepdims=True) + np.float32(1e-9))
    return ti.astype(np.int64), sw.astype(np.float32)


def _dispatch(ti):
    """Slot assignment: (token, k) pair -> (expert, position-in-expert)."""
    Tn = ti.shape[0]
    flat_e = ti.ravel()
    order = np.argsort(flat_e, kind="stable")
    cnt = np.bincount(flat_e, minlength=E)
    starts = np.concatenate([[0], np.cumsum(cnt)[:-1]])
    ranks = np.arange(Tn * TOPK) - starts[flat_e[order]]
    pos = np.empty(Tn * TOPK, np.int64)
    pos[order] = ranks
    return flat_e, pos, cnt, starts, order


def _gelu_exact(v):
    try:
        from scipy.special import erf
        return 0.5 * v * (1.0 + erf(v / np.sqrt(2.0)))
    except ImportError:  # tanh approximation fallback (overflow tokens only)
        return 0.5 * v * (1.0 + np.tanh(
            0.7978845608028654 * (v + 0.044715 * v ** 3)))


def kernel(**inputs) -> np.ndarray:
    global LAST
    x = np.asarray(inputs["x"], np.float32)
    gate_w = np.asarray(inputs["gate_w"], np.float32)
    gate_b = np.asarray(inputs["gate_b"], np.float32)
    w1 = np.asarray(inputs["w1"], np.float32)
    b1 = np.asarray(inputs["b1"], np.float32)
    w2 = np.asarray(inputs["w2"], np.float32)
    b2 = np.asarray(inputs["b2"], np.float32)

    B, S, D_ = x.shape
    flat = x.reshape(-1, D_)
    Tn = flat.shape[0]

    ti, sw = _route(flat, gate_w, gate_b)
    flat_e, pos, cnt, starts, order = _dispatch(ti)

    # Capacity factor 1.0: each core processes exactly T*K/E token slots (the
    # SPMD program is uniform, so every core pays the max expert's cost —
    # capping at the mean keeps the device critical path balanced). The few
    # overflow tokens of the hottest experts are combined on the host in fp32.
    cap = (Tn * TOPK // E + P - 1) // P * P
    C = ((int(cnt.max()) + P - 1) // P) * P
    C = max(min(C, cap), P)

    xT_bf = np.ascontiguousarray(flat.T).astype(_BF16)  # [D, T]
    sw_flat = sw.ravel()

    in_maps = []
    overflow = []
    for e in range(E):
        pairs_all = order[starts[e]:starts[e] + cnt[e]]
        pairs = pairs_all[:C]
        if cnt[e] > C:
            overflow.append((e, pairs_all[C:]))
        n_e = len(pairs)
        toks = pairs // TOPK
        xt_e = np.zeros((D, C), _BF16)
        xt_e[:, :n_e] = xT_bf[:, toks]
        sc_e = np.zeros((C,), np.float32)
        sc_e[:n_e] = sw_flat[pairs]
        KO, FO = D // P, F // P
        w1_tiled = np.ascontiguousarray(
            w1[e].astype(_BF16).reshape(KO, P, FO, P).transpose(2, 1, 0, 3)
        )
        in_maps.append({
            "xt": xt_e,
            "w1t": w1_tiled,
            "w2": w2[e].astype(_BF16),
            "b1p": np.ascontiguousarray(b1[e].reshape(F // P, P).T),
            "b2r": np.ascontiguousarray(
                np.broadcast_to(b2[e], (P, D))
            ).astype(np.float32),
            "scp": np.ascontiguousarray(sc_e.reshape(C // P, P).T),
        })

    nc = _nc_cache.get(C)
    if nc is None:
        nc = _build_moe_core(C)
        _nc_cache[C] = nc

    LAST = run_bass_kernel_spmd(nc, in_maps, core_ids=list(range(E)))
    Yall = np.stack([np.asarray(LAST.results[i]["y"]) for i in range(E)])

    # Combine: device slots via two gathers; host fp32 FFN for overflow.
    in_cap = pos < C
    contrib = np.zeros((Tn * TOPK, D_), np.float32)
    idx = np.nonzero(in_cap)[0]
    contrib[idx] = Yall[flat_e[idx], pos[idx]]
    out = contrib[0::TOPK] + contrib[1::TOPK]
    for e, over in overflow:
        toks = over // TOPK
        h = _gelu_exact(flat[toks] @ w1[e] + b1[e])
        y_e = h @ w2[e] + b2[e]
        out[toks] += sw_flat[over][:, None] * y_e
    return out.reshape(B, S, D_).astype(np.float32)
